# revision 12
# baseline (speedup 1.0000x reference)
"""Trainium2 Bass kernel for the BayesianBeliefNetwork block (8-core SPMD).

Math (see problem reference):
  h    = LayerNorm(x)*gamma + beta                          [B,S,H]
  ev   = sigmoid(mean_s(h @ W_ve.T + b_ve))                 [B,V]
  post = belief-prop(ev, parents, var_emb, cpt_emb)         [B,V]  (5 iters)
  out  = [h, post] @ W_out.T + b_out + x                    [B,S,H]

Sharding: data-parallel over the B*S = 8192 tokens; core c owns 1024 tokens
(batch b = c//2, sequence half c%2).  Parameters replicated.  The per-batch
evidence is completed with a pairwise AllReduce of the [V] partial logit
sums between the two cores sharing a batch.

Device layout: transposed - H on partitions, tokens on the free axis.
LayerNorm folds into the matmul epilogue.

Precision: the dominant W1 matmul runs MIXED - contraction chunks 0..9 in
bf16 (weights pre-scaled x64) and chunks 10..15 as fp8e4m3 DoubleRow pairs
(weights x16, x x4, so both paths accumulate 64*W*x into the same PSUM
group; measured end-to-end max-rel 1.43e-2 vs the 2e-2 gate).  The column
sums r1 are taken over the QUANTIZED weights on the host so the -r1*mu
correction is exact.

LN stats are single-row: the evidence matmul's stationary gets an extra
64*sum(x) column (row 10 of the logits PSUM), sum(x^2) comes from a
1-column fp8 DoubleRow matmul over x^2 = (x8/8)*x8, and rstd/64 (the /64
de-scales the matmul epilogue; folded into the Exp bias) + mu*rstd rows
are broadcast to [128,T] with tiny ones-stationary matmuls.  PSUM fits in
exactly 8 banks: 4 acc (t-paired j pipeline) + 2 logits + 2 rotating
(sq -> rstd/murstd broadcast -> belief-prop -> ccol).

Schedule: x8/xbf/w1 ride batched pair-triggers on the sync+scalar HWDGE
rings (DMA trigger instrs cost ~0.65us on the issuing queue); evidence
(lg+sq) runs first on the PE so the tiny AllReduce (a global barrier,
~7us + core skew) issues by ~15us and belief-prop + ccol land mid-stream;
output chunks 0..13 stage in SBUF and take a late ACT +ccol pass, the
last pair folds ccol into the evict and DMAs 4-way split.
"""

import numpy as np
import ml_dtypes

import concourse.bass as bass
import concourse.tile as tile
from concourse import bacc, mybir
from concourse.bass_utils import run_bass_kernel_spmd

F32 = mybir.dt.float32
BF16 = mybir.dt.bfloat16
F8 = mybir.dt.float8e4
PM = mybir.MatmulPerfMode
OP = mybir.AluOpType
AF = mybir.ActivationFunctionType

H = 2048
V = 10
D4 = 512
B = 4
S = 2048
N_CORES = 8
T = (B * S) // N_CORES          # 1024 tokens per core
NCH = H // 128                  # 16 h-chunks
TB = T // 512                   # 2 token halves of 512
LN_EPS = 1e-5
N_ITERS = 5
K_BF = 10                       # bf16 contraction chunks (rest fp8 DR pairs)
NDR = (NCH - K_BF) // 2         # 3 DoubleRow pair-groups
FOLD_P = 7                      # stage pairs 0..6 take the late ACT +ccol
LN64 = float(np.log(64.0))

# param32 mega-tensor column map
PC_NR1 = 0            # [128,16]
PC_BOUT = 16          # [128,16]
PC_CPT = 32           # [10,512]
PC_PFT = 544          # [10,10]
PC_RVE = 554          # [10,1]
PC_BVE = 555          # [10,1]
PC_HASP = 556         # [10,1]
PC_N = 557
# param16 (bf16) columns
PB_W2T = 0            # [10,2048]
PB_VAR = 2048         # [10,512]
PB_N = 2560

_PROG = None


def build_program():
    nc = bacc.Bacc("TRN2", target_bir_lowering=False, debug=False,
                   num_devices=N_CORES)

    xbf_d = nc.dram_tensor("xbfT", [128, NCH, T], BF16, kind="ExternalInput").ap()
    x8_d = nc.dram_tensor("x8T", [128, NCH, T], F8, kind="ExternalInput").ap()
    x28_d = nc.dram_tensor("x28T", [128, NCH, T], F8, kind="ExternalInput").ap()
    w1b_d = nc.dram_tensor("w1b", [128, 8, 2, K_BF, 128], BF16,
                           kind="ExternalInput").ap()
    w18_d = nc.dram_tensor("w18", [128, 2, 4, 2, NDR, 2, 128], F8,
                           kind="ExternalInput").ap()
    wve_d = nc.dram_tensor("wve8", [128, NCH // 2 + 1, 2, 48], F8,
                           kind="ExternalInput").ap()
    p32_d = nc.dram_tensor("p32", [128, PC_N], F32, kind="ExternalInput").ap()
    p16_d = nc.dram_tensor("p16", [128, PB_N], BF16, kind="ExternalInput").ap()
    out_d = nc.dram_tensor("outT", [H, T], BF16, kind="ExternalOutput").ap()

    with tile.TileContext(nc) as tc:
        with (
            tc.tile_pool(name="pc", bufs=1) as pc,
            tc.tile_pool(name="px8", bufs=4) as px8,
            tc.tile_pool(name="px2", bufs=4) as px2,
            tc.tile_pool(name="pxb", bufs=8) as pxb,
            tc.tile_pool(name="pwb", bufs=8) as pwb,
            tc.tile_pool(name="pw8", bufs=2) as pw8,
            tc.tile_pool(name="pst", bufs=8) as pst,
            tc.tile_pool(name="pacc", bufs=4, space="PSUM") as pacc,
            tc.tile_pool(name="pev", bufs=2, space="PSUM") as pev,
            tc.tile_pool(name="pdram", bufs=1, space="DRAM") as pdram,
        ):
            # ---- DMAs: gpsimd ring (params), sync+scalar rings (bulk) ----
            wve_sb = pc.tile([128, NCH // 2 + 1, 2, 48], F8)
            nc.gpsimd.dma_start(out=wve_sb[:], in_=wve_d[:])
            p32_sb = pc.tile([128, PC_N], F32)
            nc.gpsimd.dma_start(out=p32_sb[:], in_=p32_d[:])
            p16_sb = pc.tile([128, PB_N], BF16)
            nc.gpsimd.dma_start(out=p16_sb[:], in_=p16_d[:])
            w18_sb = []
            for g in range(2):
                w18g = pw8.tile([128, 4, 2, NDR, 2, 128], F8, tag="w18",
                                bufs=2, name=f"w18g{g}")
                w18_sb.append(w18g)

            x8q = []
            for q in range(4):
                x8t = px8.tile([128, 4, T], F8, tag="x8", bufs=4,
                               name=f"x8q{q}")
                x8q.append(x8t)
            xbfp = []
            for p in range(8):
                xb = pxb.tile([128, 2, T], BF16, tag="xbf", bufs=8,
                              name=f"xbfp{p}")
                xbfp.append(xb)
            w1bp = []
            for p in range(8):
                wb = pwb.tile([128, 2, K_BF, 128], BF16, tag="w1b", bufs=8,
                              name=f"w1bp{p}")
                w1bp.append(wb)
            x28q = []
            for q in range(4):
                x2t = px2.tile([128, 4, T], F8, tag="x2", bufs=4,
                               name=f"x28q{q}")
                x28q.append(x2t)
            # x8 + x28 first (evidence/stats path), then xbf, w1b trail
            # (weights for chunk-pair p are not needed until ~6.6us * 2p).
            nc.sync.dma_start(out=x8q[0][:], in_=x8_d[:, 0:4, :])
            nc.sync.dma_start(out=x8q[1][:], in_=x8_d[:, 4:8, :])
            nc.scalar.dma_start(out=x8q[2][:], in_=x8_d[:, 8:12, :])
            nc.scalar.dma_start(out=x8q[3][:], in_=x8_d[:, 12:16, :])
            nc.sync.dma_start(out=x28q[0][:], in_=x28_d[:, 0:4, :])
            nc.sync.dma_start(out=x28q[1][:], in_=x28_d[:, 4:8, :])
            nc.scalar.dma_start(out=x28q[2][:], in_=x28_d[:, 8:12, :])
            nc.scalar.dma_start(out=x28q[3][:], in_=x28_d[:, 12:16, :])
            nc.sync.dma_start(out=w18_sb[0][:], in_=w18_d[:, 0])
            nc.scalar.dma_start(out=w18_sb[1][:], in_=w18_d[:, 1])
            for p in range(4):
                nc.sync.dma_start(out=xbfp[p][:], in_=xbf_d[:, 2 * p:2 * p + 2, :])
            for p in range(4, 8):
                nc.scalar.dma_start(out=xbfp[p][:], in_=xbf_d[:, 2 * p:2 * p + 2, :])
            # w1b pairs 0-1 immediately; pairs 2-7 are released inside the
            # j-loop (WAW memset gate) so their 4MB doesn't race x/xbf for
            # HBM bandwidth during the startup window.
            nc.sync.dma_start(out=w1bp[0][:], in_=w1b_d[:, 0])
            nc.scalar.dma_start(out=w1bp[1][:], in_=w1b_d[:, 1])

            def release_w1b(p):
                nc.vector.memset(w1bp[p][0:1, 0:1, 0:1, 0:1], 0.0)
                eng = nc.sync if p % 2 == 0 else nc.scalar
                eng.dma_start(out=w1bp[p][:], in_=w1b_d[:, p])

            # ---- small SBUF constants ----
            ones_sb = pc.tile([128, 128], BF16)
            nc.vector.memset(ones_sb[:], 1.0)
            eps_ln = pc.tile([33, 1], F32)
            nc.vector.memset(eps_ln[:], LN_EPS)
            nln64 = pc.tile([33, 1], F32)
            nc.vector.memset(nln64[:], -LN64)
            eps_pn = pc.tile([V, 1], F32)
            nc.vector.memset(eps_pn[:], 1e-16)

            # ---- PE warm-up: trip the HAM clock gate (rotates pev slots) ----
            for i in range(24):
                warm = pev.tile([128, 512], F32, tag="sq", bufs=2,
                                name=f"warm{i}")
                nc.tensor.matmul(warm[:, 0:128], ones_sb[:], ones_sb[:],
                                 start=True, stop=True)

            # ---- evidence + stats matmuls (DoubleRow fp8) ----
            # lg rows 0..9 = 64*Wveg^T x ; row 10 = 64*sum(x)
            lgs = [pev.tile([48, 512], F32, tag="lg", bufs=2, name=f"lg{t}")
                   for t in range(TB)]
            sqr = [pev.tile([48, 512], F32, tag="sq", bufs=2, name=f"sqr{t}")
                   for t in range(TB)]
            for p2 in range(NCH // 2):
                q, o = p2 // 2, (p2 % 2) * 2
                for t in range(TB):
                    sl = slice(t * 512, (t + 1) * 512)
                    nc.tensor.matmul(lgs[t][:], wve_sb[:, p2, :, :],
                                     x8q[q][:, o:o + 2, sl],
                                     start=(p2 == 0), stop=(p2 == 7),
                                     perf_mode=PM.DoubleRow)
            for p2 in range(NCH // 2):
                q, o = p2 // 2, (p2 % 2) * 2
                for t in range(TB):
                    sl = slice(t * 512, (t + 1) * 512)
                    nc.tensor.matmul(sqr[t][:], wve_sb[:, 8, :, :],
                                     x28q[q][:, o:o + 2, sl],
                                     start=(p2 == 0), stop=(p2 == 7),
                                     perf_mode=PM.DoubleRow)

            # ---- LN stats on single rows ----
            R = slice(32, 33)
            mu_row = pc.tile([33, T], BF16)
            mu2_row = pc.tile([33, T], F32)
            var_row = pc.tile([33, T], F32)
            rstd64_row = pc.tile([33, T], BF16)
            murstd_row = pc.tile([33, T], BF16)
            for t in range(TB):
                sl = slice(t * 512, (t + 1) * 512)
                nc.vector.tensor_scalar_mul(mu_row[R, sl], lgs[t][32:33, :],
                                            1.0 / (64.0 * H))
            nc.vector.tensor_mul(mu2_row[R, :], mu_row[R, :], mu_row[R, :])
            for t in range(TB):
                sl = slice(t * 512, (t + 1) * 512)
                nc.vector.scalar_tensor_tensor(
                    out=var_row[R, sl], in0=sqr[t][32:33, :], scalar=0.5 / H,
                    in1=mu2_row[R, sl], op0=OP.mult, op1=OP.subtract)
            # rstd/64 = exp(-0.5*ln(var+eps) - ln 64); /64 de-scales the
            # x64 accumulated matmul at the evict multiply.
            nc.scalar.activation(var_row[R, :], var_row[R, :], AF.Ln,
                                 bias=eps_ln[32:33, :])
            nc.scalar.activation(rstd64_row[R, :], var_row[R, :], AF.Exp,
                                 bias=nln64[32:33, :], scale=-0.5)
            nc.vector.scalar_tensor_tensor(
                out=murstd_row[R, :], in0=mu_row[R, :], scalar=64.0,
                in1=rstd64_row[R, :], op0=OP.mult, op1=OP.mult)

            # ---- broadcast rstd64/murstd rows to [128, T] via PE ----
            rstd_bc = pc.tile([128, T], BF16)
            murstd_bc = pc.tile([128, T], BF16)
            for t in range(TB):
                sl = slice(t * 512, (t + 1) * 512)
                bcp = pev.tile([128, 512], F32, tag="sq", bufs=2,
                               name=f"rstdps{t}")
                nc.tensor.matmul(bcp[:], ones_sb[32:33, :], rstd64_row[R, sl],
                                 start=True, stop=True)
                nc.vector.tensor_copy(rstd_bc[:, sl], bcp[:])
            for t in range(TB):
                sl = slice(t * 512, (t + 1) * 512)
                bcp = pev.tile([128, 512], F32, tag="sq", bufs=2,
                               name=f"murps{t}")
                nc.tensor.matmul(bcp[:], ones_sb[32:33, :], murstd_row[R, sl],
                                 start=True, stop=True)
                nc.vector.tensor_copy(murstd_bc[:, sl], bcp[:])

            # ---- evidence partials + AllReduce ----
            # ev[v] = sum_t lg[v,t]*rstd[t] - rve[v]*sum_t murstd[t]
            ev_acc = pc.tile([V, TB], F32)
            rv10 = pc.tile([V, 1], F32)
            junk10 = pc.tile([V, T], BF16)
            nc.vector.scalar_tensor_tensor(
                out=junk10[:], in0=murstd_bc[0:V, :],
                scalar=p32_sb[0:V, PC_RVE:PC_RVE + 1],
                in1=murstd_bc[0:V, :], op0=OP.mult, op1=OP.bypass,
                accum_out=rv10[:])
            lgjunk = pc.tile([V, 512], F32)
            for t in range(TB):
                sl = slice(t * 512, (t + 1) * 512)
                nc.vector.scalar_tensor_tensor(
                    out=lgjunk[:], in0=lgs[t][0:V, :], scalar=1.0,
                    in1=rstd_bc[0:V, sl], op0=OP.mult, op1=OP.mult,
                    accum_out=ev_acc[:, t:t + 1])
            ev_sb = pc.tile([V, 1], F32)
            nc.vector.tensor_add(ev_sb[:], ev_acc[:, 0:1], ev_acc[:, 1:2])
            nc.vector.tensor_sub(ev_sb[:], ev_sb[:], rv10[:])

            cc_in = pdram.tile([V, 1], F32)
            cc_out = pdram.tile([V, 1], F32)
            nc.gpsimd.dma_start(out=cc_in[:], in_=ev_sb[:])
            nc.gpsimd.collective_compute(
                "AllReduce", OP.add,
                replica_groups=[[0, 1], [2, 3], [4, 5], [6, 7]],
                ins=[cc_in.opt()], outs=[cc_out.opt()])
            cc_sb = pc.tile([V, 1], F32)
            nc.gpsimd.dma_start(out=cc_sb[:], in_=cc_out[:])

            # ---- belief propagation (tiny; overlaps the main stream) ----
            SIG_C = (0.2499968877665068, -0.020805674064028827,
                     2.0168972875466143e-03, -1.499637664404622e-04)
            SIG3 = (0.24945, -0.0187)

            def emit_sigmoid_poly(out, x, tag):
                c1, c3, c5, c7 = SIG_C
                x2p = pc.tile([V, 1], F32, name=f"sx2_{tag}")
                nc.vector.tensor_mul(x2p[:], x[:], x[:])
                p = pc.tile([V, 1], F32, name=f"sp_{tag}")
                nc.vector.tensor_scalar(p[:], x2p[:], c7, c5, op0=OP.mult,
                                        op1=OP.add)
                nc.vector.tensor_mul(p[:], p[:], x2p[:])
                nc.vector.tensor_scalar(p[:], p[:], c3, None, op0=OP.add)
                nc.vector.tensor_mul(p[:], p[:], x2p[:])
                nc.vector.tensor_scalar(p[:], p[:], c1, None, op0=OP.add)
                nc.vector.tensor_mul(p[:], p[:], x[:])
                nc.vector.tensor_scalar(out[:], p[:], 0.5, None, op0=OP.add)

            bp = {}

            def emit_bp_pre():
                ev_arg = pc.tile([V, 1], F32)
                nc.vector.tensor_scalar_mul(ev_arg[:], cc_sb[:], 1.0 / S)
                nc.vector.tensor_add(ev_arg[:], ev_arg[:],
                                     p32_sb[0:V, PC_BVE:PC_BVE + 1])
                ev0 = pc.tile([V, 1], F32)
                emit_sigmoid_poly(ev0, ev_arg, "ev")
                m1 = pc.tile([V, 1], F32)
                nc.vector.tensor_scalar(m1[:], ev0[:], 0.1, None, op0=OP.is_gt)
                mask = pc.tile([V, 1], F32)
                nc.vector.tensor_scalar(mask[:], ev0[:], 0.9, None,
                                        op0=OP.is_lt)
                nc.vector.tensor_mul(mask[:], mask[:], m1[:])
                nc.vector.tensor_scalar(mask[:], mask[:],
                                        p32_sb[0:V, PC_HASP:PC_HASP + 1],
                                        None, op0=OP.mult)
                probs = pc.tile([V, 1], F32)
                nc.vector.tensor_copy(probs[:], ev0[:])
                bp.update(mask=mask, probs=probs)

            def emit_bp_iter(it):
                mask, probs = bp["mask"], bp["probs"]
                lhsT = pc.tile([V, V], BF16, name=f"lhsT{it}")
                nc.vector.tensor_scalar(lhsT[:],
                                        p32_sb[0:V, PC_PFT:PC_PFT + V],
                                        probs[:, 0:1], None, op0=OP.mult)
                pe_ps = pev.tile([V, 512], F32, tag="lg", bufs=2,
                                 name=f"pe{it}")
                nc.tensor.matmul(pe_ps[:], lhsT[:],
                                 p16_sb[0:V, PB_VAR:PB_VAR + D4],
                                 start=True, stop=True)
                pe_sb = pc.tile([V, D4], F32, tag="bscr", bufs=4,
                                name=f"pe_sb{it}")
                nc.vector.tensor_copy(pe_sb[:], pe_ps[:])
                bscr = pc.tile([V, D4], F32, tag="bscr", bufs=4,
                               name=f"bscr{it}")
                dot = pc.tile([V, 1], F32, name=f"dot{it}")
                nc.vector.scalar_tensor_tensor(
                    out=bscr[:], in0=pe_sb[:], scalar=1.0,
                    in1=p32_sb[0:V, PC_CPT:PC_CPT + D4],
                    op0=OP.mult, op1=OP.mult, accum_out=dot[:])
                bscr2 = pc.tile([V, D4], F32, tag="bscr", bufs=4,
                                name=f"bscr2{it}")
                sqa = pc.tile([V, 1], F32, name=f"sqa{it}")
                nc.vector.scalar_tensor_tensor(
                    out=bscr2[:], in0=pe_sb[:], scalar=1.0, in1=pe_sb[:],
                    op0=OP.mult, op1=OP.mult, accum_out=sqa[:])
                nc.scalar.activation(sqa[:], sqa[:], AF.Sqrt, bias=eps_pn[:])
                ipn = pc.tile([V, 1], F32, name=f"ipn{it}")
                nc.vector.reciprocal(ipn[:], sqa[:])
                s = pc.tile([V, 1], F32, name=f"s{it}")
                nc.vector.tensor_mul(s[:], dot[:], ipn[:])
                sx2 = pc.tile([V, 1], F32, name=f"3x2_{it}")
                nc.vector.tensor_mul(sx2[:], s[:], s[:])
                sp = pc.tile([V, 1], F32, name=f"3p_{it}")
                nc.vector.tensor_scalar(sp[:], sx2[:], SIG3[1], SIG3[0],
                                        op0=OP.mult, op1=OP.add)
                nc.vector.tensor_mul(sp[:], sp[:], s[:])
                cond = pc.tile([V, 1], F32, name=f"cond{it}")
                nc.vector.tensor_scalar(cond[:], sp[:], 0.5, None, op0=OP.add)
                delta = pc.tile([V, 1], F32, name=f"delta{it}")
                nc.vector.tensor_sub(delta[:], cond[:], probs[:])
                nc.vector.scalar_tensor_tensor(
                    out=probs[:], in0=delta[:], scalar=mask[:, 0:1],
                    in1=probs[:], op0=OP.mult, op1=OP.add)

            def emit_ccol():
                probs_bf = pc.tile([V, 1], BF16)
                nc.vector.tensor_copy(probs_bf[:], bp["probs"][:])
                ccol_ps = pev.tile([128, 512], F32, tag="lg", bufs=2,
                                   name="ccol_ps")
                for c in range(NCH):
                    nc.tensor.matmul(ccol_ps[:, c:c + 1],
                                     p16_sb[0:V, PB_W2T + c * 128:
                                            PB_W2T + (c + 1) * 128],
                                     probs_bf[:], start=True, stop=True)
                ccol_sb = pc.tile([128, NCH], F32)
                nc.vector.tensor_add(ccol_sb[:], ccol_ps[:, 0:NCH],
                                     p32_sb[:, PC_BOUT:PC_BOUT + NCH])
                bp["ccol"] = ccol_sb

            # ---- main stream: t-paired accumulation, mixed bf16 + fp8 DR ----
            pend = {}

            def emit_evict(j, t, acc, stage):
                sl = slice(t * 512, (t + 1) * 512)
                s3 = pst.tile([128, 512], BF16, tag="s3", bufs=3,
                              name=f"s3_{j}_{t}")
                nc.vector.scalar_tensor_tensor(
                    out=s3[:], in0=acc[:], scalar=1.0, in1=rstd_bc[:, sl],
                    op0=OP.mult, op1=OP.mult)
                s4 = pst.tile([128, 512], BF16, tag="s4", bufs=3,
                              name=f"s4_{j}_{t}")
                nc.vector.scalar_tensor_tensor(
                    out=s4[:], in0=murstd_bc[:, sl],
                    scalar=p32_sb[:, PC_NR1 + j:PC_NR1 + j + 1],
                    in1=s3[:], op0=OP.mult, op1=OP.add)
                xb = xbfp[j // 2][:, j % 2, sl]
                if j // 2 >= FOLD_P:
                    nc.vector.scalar_tensor_tensor(
                        out=stage[:, j % 2, sl], in0=xb,
                        scalar=bp["ccol"][:, j:j + 1],
                        in1=s4[:], op0=OP.add, op1=OP.add)
                else:
                    nc.vector.tensor_add(stage[:, j % 2, sl], xb, s4[:])

            def pair_out_ap(p):
                return out_d[p * 256:(p + 1) * 256, :].rearrange(
                    "(i p) t -> p i t", p=128)

            def emit_pair_dma(p, stage, nsp, ring):
                ap = pair_out_ap(p)
                for qq in range(nsp):
                    p0 = qq * (128 // nsp)
                    p1 = p0 + (128 // nsp)
                    eng = ring[qq % len(ring)]
                    eng.dma_start(out=ap[p0:p1], in_=stage[p0:p1])

            def emit_late_out():
                for p in range(FOLD_P):
                    stage = pend.pop(p)
                    nc.scalar.activation(
                        stage[:, 0, :], stage[:, 0, :], AF.Identity,
                        bias=bp["ccol"][:, 2 * p:2 * p + 1])
                    nc.vector.scalar_tensor_tensor(
                        out=stage[:, 1, :], in0=stage[:, 1, :],
                        scalar=bp["ccol"][:, 2 * p + 1:2 * p + 2],
                        in1=stage[:, 1, :], op0=OP.add, op1=OP.bypass)
                    ring = [nc.scalar] if p % 2 == 0 else [nc.sync]
                    emit_pair_dma(p, stage, 1, ring)

            for j in range(NCH):
                if j % 2 == 0:
                    stage = pst.tile([128, 2, T], BF16, tag="stage", bufs=8,
                                     name=f"stage{j // 2}")
                accs = [pacc.tile([128, 512], F32, tag="acc", bufs=4,
                                  name=f"acc{j}_{t}") for t in range(TB)]
                for hin in range(K_BF):
                    wst = w1bp[j // 2][:, j % 2, hin, :]
                    for t in range(TB):
                        sl = slice(t * 512, (t + 1) * 512)
                        nc.tensor.matmul(
                            accs[t][:], wst, xbfp[hin // 2][:, hin % 2, sl],
                            start=(hin == 0), stop=False)
                for m in range(NDR):
                    p2 = 5 + m
                    q, o = p2 // 2, (p2 % 2) * 2
                    wst = w18_sb[j // 8][:, (j // 2) % 4, j % 2, m, :, :]
                    for t in range(TB):
                        sl = slice(t * 512, (t + 1) * 512)
                        nc.tensor.matmul(
                            accs[t][:], wst, x8q[q][:, o:o + 2, sl],
                            start=False, stop=(m == NDR - 1),
                            perf_mode=PM.DoubleRow)
                    if j == 4 and m == NDR - 1:
                        emit_bp_pre()
                        emit_bp_iter(0)
                for t in range(TB):
                    emit_evict(j, t, accs[t], stage)
                if j % 2 == 1 and j // 2 < FOLD_P:
                    pend[j // 2] = stage
                if j % 2 == 1 and j // 2 == FOLD_P:
                    emit_pair_dma(FOLD_P, stage, 4, [nc.sync, nc.scalar])
                if j < 6:
                    release_w1b(j + 2)
                if j in (5, 6, 7, 8):
                    emit_bp_iter(j - 4)
                if j == 9:
                    emit_ccol()
                if j == 13:
                    emit_late_out()

    nc.compile()
    return nc


def _host_prep(hidden_states, gamma, beta, W_ve, b_ve, var_emb, cpt_emb,
               W_out, b_out, parents):
    f32 = np.float32
    bf16 = ml_dtypes.bfloat16
    fp8 = ml_dtypes.float8_e4m3
    x = np.asarray(hidden_states, f32).reshape(B * S, H)
    gamma = np.asarray(gamma, f32)
    beta = np.asarray(beta, f32)
    W_ve = np.asarray(W_ve, f32)
    b_ve = np.asarray(b_ve, f32)
    var_emb = np.asarray(var_emb, f32)
    cpt_emb = np.asarray(cpt_emb, f32)
    W_out = np.asarray(W_out, f32)
    b_out = np.asarray(b_out, f32)
    parents = np.asarray(parents)

    W1 = W_out[:, :H]
    W1g = W1 * gamma[None, :]
    # quantized W1g: chunks < K_BF as bf16(64*W), rest fp8(16*W)
    w1b = np.ascontiguousarray(
        (64.0 * W1g).T.reshape(NCH, 128, 8, 2, 128)[:K_BF]
        .transpose(1, 2, 3, 0, 4)).astype(bf16)
    # w18[p, g, q, jj, m, i, c] = fp8(16*W1g[j*128+c, (K_BF+2m+i)*128+p])
    w18f = (16.0 * W1g).T.reshape(NCH, 128, 2, 4, 2, 128)[K_BF:]
    w18f = w18f.reshape(NDR, 2, 128, 2, 4, 2, 128)
    w18 = np.ascontiguousarray(w18f.transpose(2, 3, 4, 5, 0, 1, 6)).astype(fp8)
    # exact column sums of the quantized matrix (true scale)
    wq_eff = np.empty_like(W1g)
    for ch in range(NCH):
        blk = W1g[:, ch * 128:(ch + 1) * 128]
        if ch < K_BF:
            wq_eff[:, ch * 128:(ch + 1) * 128] = \
                (64.0 * blk).astype(bf16).astype(f32) / 64.0
        else:
            wq_eff[:, ch * 128:(ch + 1) * 128] = \
                (16.0 * blk).astype(fp8).astype(f32) / 16.0
    r1 = wq_eff.sum(axis=1)                                  # [H]
    Wveg = W_ve * gamma[None, :]
    wve8 = np.zeros((128, NCH // 2 + 1, 2, 48), fp8)
    wq = (16.0 * Wveg.T).astype(fp8)                         # [H, V]
    wve8[:, :8, :, :V] = wq.reshape(NCH // 2, 2, 128, V).transpose(2, 0, 1, 3)
    wve8[:, :8, :, 32] = np.float32(16.0)                    # 64*sum(x) row
    wve8[:, 8, :, 32] = np.float32(1.0)                      # sq ones column
    rve = (wq.astype(f32) / 16.0).sum(axis=0)                # [V]

    p32 = np.zeros((128, PC_N), f32)
    p32[:, PC_NR1:PC_NR1 + NCH] = (-r1).reshape(NCH, 128).T
    p32[:, PC_BOUT:PC_BOUT + NCH] = (b_out + W1 @ beta).reshape(NCH, 128).T
    icn = 1.0 / np.maximum(np.sqrt((cpt_emb * cpt_emb).sum(axis=1)), 1e-8)
    p32[:V, PC_CPT:PC_CPT + D4] = cpt_emb * icn[:, None]
    p32[:V, PC_PFT:PC_PFT + V] = parents.T.astype(f32)
    p32[:V, PC_RVE] = rve
    p32[:V, PC_BVE] = b_ve + W_ve @ beta
    p32[:V, PC_HASP] = (parents.sum(axis=1) > 0).astype(f32)
    p16 = np.zeros((128, PB_N), bf16)
    p16[:V, PB_W2T:PB_W2T + H] = np.ascontiguousarray(W_out[:, H:].T)
    p16[:V, PB_VAR:PB_VAR + D4] = var_emb.astype(bf16)

    shared = dict(w1b=w1b, w18=w18, wve8=wve8, p32=p32, p16=p16)
    in_maps = []
    for c in range(N_CORES):
        xs = x[c * T:(c + 1) * T, :]
        xbfT = np.ascontiguousarray(
            xs.T.reshape(NCH, 128, T).transpose(1, 0, 2)).astype(bf16)
        x8T = np.ascontiguousarray(
            (4.0 * xs).T.reshape(NCH, 128, T).transpose(1, 0, 2)).astype(fp8)
        x28T = np.ascontiguousarray(
            (2.0 * xs * xs).T.reshape(NCH, 128, T).transpose(1, 0, 2)
        ).astype(fp8)
        in_maps.append(dict(shared, xbfT=xbfT, x8T=x8T, x28T=x28T))
    return in_maps


def kernel(**inputs):
    global _PROG
    if _PROG is None:
        _PROG = build_program()
    nc = _PROG
    in_maps = _host_prep(**inputs)
    res = run_bass_kernel_spmd(nc, in_maps, list(range(N_CORES)))
    out = np.empty((B * S, H), np.float32)
    for c in range(N_CORES):
        out[c * T:(c + 1) * T, :] = \
            np.asarray(res.results[c]["outT"]).astype(np.float32).T
    return out.reshape(B, S, H)


# revision 14
# speedup vs baseline: 1.0459x; 1.0459x over previous
"""Trainium2 Bass kernel for the BayesianBeliefNetwork block (8-core SPMD).

Math (see problem reference):
  h    = LayerNorm(x)*gamma + beta                          [B,S,H]
  ev   = sigmoid(mean_s(h @ W_ve.T + b_ve))                 [B,V]
  post = belief-prop(ev, parents, var_emb, cpt_emb)         [B,V]  (5 iters)
  out  = [h, post] @ W_out.T + b_out + x                    [B,S,H]

Sharding: data-parallel over the B*S = 8192 tokens; core c owns 1024 tokens
(batch b = c//2, sequence half c%2).  Parameters replicated.  The per-batch
evidence is completed with a pairwise AllReduce of the [V] partial logit
sums between the two cores sharing a batch.

Device layout: transposed - H on partitions, tokens on the free axis.
LayerNorm folds into the matmul epilogue.

Precision: the dominant W1 matmul runs MIXED - contraction chunks 0..9 in
bf16 (weights pre-scaled x64) and chunks 10..15 as fp8e4m3 DoubleRow pairs
(weights x16, x x4, so both paths accumulate 64*W*x into the same PSUM
group; measured end-to-end max-rel 1.43e-2 vs the 2e-2 gate).  The column
sums r1 are taken over the QUANTIZED weights on the host so the -r1*mu
correction is exact.

LN stats are single-row: the evidence matmul's stationary gets an extra
64*sum(x) column (row 10 of the logits PSUM), sum(x^2) comes from a
1-column fp8 DoubleRow matmul over x^2 = (x8/8)*x8, and rstd/64 (the /64
de-scales the matmul epilogue; folded into the Exp bias) + mu*rstd rows
are broadcast to [128,T] with tiny ones-stationary matmuls.  PSUM fits in
exactly 8 banks: 4 acc (t-paired j pipeline) + 2 logits + 2 rotating
(sq -> rstd/murstd broadcast -> belief-prop -> ccol).

Schedule: x8/xbf/w1 ride batched pair-triggers on the sync+scalar HWDGE
rings (DMA trigger instrs cost ~0.65us on the issuing queue); evidence
(lg+sq) runs first on the PE so the tiny AllReduce (a global barrier,
~7us + core skew) issues by ~15us and belief-prop + ccol land mid-stream;
output chunks 0..13 stage in SBUF and take a late ACT +ccol pass, the
last pair folds ccol into the evict and DMAs 4-way split.
"""

import numpy as np
import ml_dtypes

import concourse.bass as bass
import concourse.tile as tile
from concourse import bacc, mybir
from concourse.bass_utils import run_bass_kernel_spmd

F32 = mybir.dt.float32
BF16 = mybir.dt.bfloat16
F8 = mybir.dt.float8e4
PM = mybir.MatmulPerfMode
OP = mybir.AluOpType
AF = mybir.ActivationFunctionType

H = 2048
V = 10
D4 = 512
B = 4
S = 2048
N_CORES = 8
T = (B * S) // N_CORES          # 1024 tokens per core
NCH = H // 128                  # 16 h-chunks
TB = T // 512                   # 2 token halves of 512
LN_EPS = 1e-5
N_ITERS = 5
K_BF = 10                       # bf16 contraction chunks (rest fp8 DR pairs)
NDR = (NCH - K_BF) // 2         # 3 DoubleRow pair-groups
FOLD_P = 7                      # stage pairs 0..6 take the late ACT +ccol
LN64 = float(np.log(64.0))

# param32 mega-tensor column map
PC_NR1 = 0            # [128,16]
PC_BOUT = 16          # [128,16]
PC_CPT = 32           # [10,512]
PC_PFT = 544          # [10,10]
PC_RVE = 554          # [10,1]
PC_BVE = 555          # [10,1]
PC_HASP = 556         # [10,1]
PC_N = 557
# param16 (bf16) columns
PB_W2T = 0            # [10,2048]
PB_VAR = 2048         # [10,512]
PB_N = 2560

_PROG = None


def build_program():
    nc = bacc.Bacc("TRN2", target_bir_lowering=False, debug=False,
                   num_devices=N_CORES)

    xbf_d = nc.dram_tensor("xbfT", [128, NCH, T], BF16, kind="ExternalInput").ap()
    x8_d = nc.dram_tensor("x8T", [128, NCH, T], F8, kind="ExternalInput").ap()
    w1b_d = nc.dram_tensor("w1b", [128, 8, 2, K_BF, 128], BF16,
                           kind="ExternalInput").ap()
    w18_d = nc.dram_tensor("w18", [128, 2, 4, 2, NDR, 2, 128], F8,
                           kind="ExternalInput").ap()
    wve_d = nc.dram_tensor("wve8", [128, NCH // 2 + 1, 2, 48], F8,
                           kind="ExternalInput").ap()
    p32_d = nc.dram_tensor("p32", [128, PC_N], F32, kind="ExternalInput").ap()
    p16_d = nc.dram_tensor("p16", [128, PB_N], BF16, kind="ExternalInput").ap()
    out_d = nc.dram_tensor("outT", [H, T], BF16, kind="ExternalOutput").ap()

    with tile.TileContext(nc) as tc:
        with (
            tc.tile_pool(name="pc", bufs=1) as pc,
            tc.tile_pool(name="px8", bufs=4) as px8,
            tc.tile_pool(name="px2", bufs=4) as px2,
            tc.tile_pool(name="pxb", bufs=8) as pxb,
            tc.tile_pool(name="pwb", bufs=8) as pwb,
            tc.tile_pool(name="pw8", bufs=2) as pw8,
            tc.tile_pool(name="pst", bufs=8) as pst,
            tc.tile_pool(name="pacc", bufs=4, space="PSUM") as pacc,
            tc.tile_pool(name="pev", bufs=2, space="PSUM") as pev,
            tc.tile_pool(name="pdram", bufs=1, space="DRAM") as pdram,
        ):
            # ---- DMAs: gpsimd ring (params), sync+scalar rings (bulk) ----
            wve_sb = pc.tile([128, NCH // 2 + 1, 2, 48], F8)
            nc.gpsimd.dma_start(out=wve_sb[:], in_=wve_d[:])
            p32_sb = pc.tile([128, PC_N], F32)
            nc.gpsimd.dma_start(out=p32_sb[:], in_=p32_d[:])
            p16_sb = pc.tile([128, PB_N], BF16)
            nc.gpsimd.dma_start(out=p16_sb[:], in_=p16_d[:])
            w18_sb = []
            for g in range(2):
                w18g = pw8.tile([128, 4, 2, NDR, 2, 128], F8, tag="w18",
                                bufs=2, name=f"w18g{g}")
                w18_sb.append(w18g)

            x8q = []
            for q in range(4):
                x8t = px8.tile([128, 4, T], F8, tag="x8", bufs=4,
                               name=f"x8q{q}")
                x8q.append(x8t)
            xbfp = []
            for p in range(8):
                xb = pxb.tile([128, 2, T], BF16, tag="xbf", bufs=8,
                              name=f"xbfp{p}")
                xbfp.append(xb)
            w1bp = []
            for p in range(8):
                wb = pwb.tile([128, 2, K_BF, 128], BF16, tag="w1b", bufs=6,
                              name=f"w1bp{p}")
                w1bp.append(wb)
            # x^2 pairs: chunks 0-9 as bf16 xbf^2 (DVE), chunks 10-15 as
            # 16*x^2 = x8^2 (gpsimd, x8 carries 4x; de-scaled by the 1/16
            # sq stationary).  No HBM cost, engines are idle there anyway.
            x2p = []
            for p in range(8):
                x2t = px2.tile([128, 2, T], BF16, tag="x2", bufs=8,
                               name=f"x2p{p}")
                x2p.append(x2t)
            # Startup HBM budget is the binding constraint (~330GB/s): only
            # what the evidence path + first chunks need goes immediately;
            # w1b pairs 2-7 / xbf pairs 5-7 / w18g1 release inside the
            # j-loop via a WAW memset gate.
            nc.sync.dma_start(out=x8q[0][:], in_=x8_d[:, 0:4, :])
            nc.sync.dma_start(out=x8q[1][:], in_=x8_d[:, 4:8, :])
            nc.scalar.dma_start(out=x8q[2][:], in_=x8_d[:, 8:12, :])
            nc.scalar.dma_start(out=x8q[3][:], in_=x8_d[:, 12:16, :])
            for p in range(3):
                nc.sync.dma_start(out=xbfp[p][:], in_=xbf_d[:, 2 * p:2 * p + 2, :])
            for p in range(3, 5):
                nc.scalar.dma_start(out=xbfp[p][:], in_=xbf_d[:, 2 * p:2 * p + 2, :])
            nc.sync.dma_start(out=w18_sb[0][:], in_=w18_d[:, 0])
            nc.sync.dma_start(out=w1bp[0][:], in_=w1b_d[:, 0])
            nc.scalar.dma_start(out=w1bp[1][:], in_=w1b_d[:, 1])

            def release_w1b(p):
                nc.vector.memset(w1bp[p][0:1, 0:1, 0:1, 0:1], 0.0)
                eng = nc.sync if p % 2 == 0 else nc.scalar
                eng.dma_start(out=w1bp[p][:], in_=w1b_d[:, p])

            def release_xbf(p):
                nc.vector.memset(xbfp[p][0:1, 0:1, 0:1], 0.0)
                eng = nc.sync if p % 2 == 0 else nc.scalar
                eng.dma_start(out=xbfp[p][:], in_=xbf_d[:, 2 * p:2 * p + 2, :])

            def release_w18g1():
                nc.vector.memset(w18_sb[1][0:1, 0:1, 0:1, 0:1, 0:1, 0:1], 0.0)
                nc.scalar.dma_start(out=w18_sb[1][:], in_=w18_d[:, 1])

            # ---- small SBUF constants ----
            ones_sb = pc.tile([128, 128], BF16)
            nc.vector.memset(ones_sb[:], 1.0)
            eps_ln = pc.tile([33, 1], F32)
            nc.vector.memset(eps_ln[:], LN_EPS)
            nln64 = pc.tile([33, 1], F32)
            nc.vector.memset(nln64[:], -LN64)
            eps_pn = pc.tile([V, 1], F32)
            nc.vector.memset(eps_pn[:], 1e-16)

            # ---- PE warm-up: trip the HAM clock gate (rotates pev slots) ----
            for i in range(24):
                warm = pev.tile([128, 512], F32, tag="sq", bufs=2,
                                name=f"warm{i}")
                nc.tensor.matmul(warm[:, 0:128], ones_sb[:], ones_sb[:],
                                 start=True, stop=True)

            # ---- evidence + stats matmuls (DoubleRow fp8) ----
            # lg rows 0..9 = 64*Wveg^T x ; row 10 = 64*sum(x)
            lgs = [pev.tile([48, 512], F32, tag="lg", bufs=2, name=f"lg{t}")
                   for t in range(TB)]
            sqr = [pev.tile([48, 512], F32, tag="sq", bufs=2, name=f"sqr{t}")
                   for t in range(TB)]
            for p2 in range(NCH // 2):
                q, o = p2 // 2, (p2 % 2) * 2
                for t in range(TB):
                    sl = slice(t * 512, (t + 1) * 512)
                    nc.tensor.matmul(lgs[t][:], wve_sb[:, p2, :, :],
                                     x8q[q][:, o:o + 2, sl],
                                     start=(p2 == 0), stop=(p2 == 7),
                                     perf_mode=PM.DoubleRow)
            s116 = pc.tile([128, 1], BF16)
            nc.vector.memset(s116[:], 1.0 / 16.0)
            for p in range(5, 8):
                q, o = p // 2, (p % 2) * 2
                nc.gpsimd.tensor_mul(x2p[p][:], x8q[q][:, o:o + 2, :],
                                     x8q[q][:, o:o + 2, :])
            for p in range(5):
                nc.vector.tensor_mul(x2p[p][:], xbfp[p][:], xbfp[p][:])
            # chunks 10-15 first (x8-derived, available earliest)
            for ch in list(range(K_BF, NCH)) + list(range(K_BF)):
                st = ones_sb[:, 0:1] if ch < K_BF else s116[:]
                first, last = (ch == K_BF), (ch == K_BF - 1)
                for t in range(TB):
                    sl = slice(t * 512, (t + 1) * 512)
                    nc.tensor.matmul(sqr[t][32:33, :], st,
                                     x2p[ch // 2][:, ch % 2, sl],
                                     start=first, stop=last)

            # ---- LN stats on single rows ----
            R = slice(32, 33)
            mu_row = pc.tile([33, T], BF16)
            mu2_row = pc.tile([33, T], F32)
            var_row = pc.tile([33, T], F32)
            rstd64_row = pc.tile([33, T], BF16)
            murstd_row = pc.tile([33, T], BF16)
            for t in range(TB):
                sl = slice(t * 512, (t + 1) * 512)
                nc.vector.tensor_scalar_mul(mu_row[R, sl], lgs[t][32:33, :],
                                            1.0 / (64.0 * H))
            nc.vector.tensor_mul(mu2_row[R, :], mu_row[R, :], mu_row[R, :])
            for t in range(TB):
                sl = slice(t * 512, (t + 1) * 512)
                nc.vector.scalar_tensor_tensor(
                    out=var_row[R, sl], in0=sqr[t][32:33, :], scalar=1.0 / H,
                    in1=mu2_row[R, sl], op0=OP.mult, op1=OP.subtract)
            # rstd/64 = exp(-0.5*ln(var+eps) - ln 64); /64 de-scales the
            # x64 accumulated matmul at the evict multiply.
            nc.scalar.activation(var_row[R, :], var_row[R, :], AF.Ln,
                                 bias=eps_ln[32:33, :])
            nc.scalar.activation(rstd64_row[R, :], var_row[R, :], AF.Exp,
                                 bias=nln64[32:33, :], scale=-0.5)
            nc.vector.scalar_tensor_tensor(
                out=murstd_row[R, :], in0=mu_row[R, :], scalar=64.0,
                in1=rstd64_row[R, :], op0=OP.mult, op1=OP.mult)

            # ---- broadcast rstd64/murstd rows to [128, T] via PE ----
            rstd_bc = pc.tile([128, T], BF16)
            murstd_bc = pc.tile([128, T], BF16)
            for t in range(TB):
                sl = slice(t * 512, (t + 1) * 512)
                bcp = pev.tile([128, 512], F32, tag="sq", bufs=2,
                               name=f"rstdps{t}")
                nc.tensor.matmul(bcp[:], ones_sb[32:33, :], rstd64_row[R, sl],
                                 start=True, stop=True)
                nc.vector.tensor_copy(rstd_bc[:, sl], bcp[:])
            for t in range(TB):
                sl = slice(t * 512, (t + 1) * 512)
                bcp = pev.tile([128, 512], F32, tag="sq", bufs=2,
                               name=f"murps{t}")
                nc.tensor.matmul(bcp[:], ones_sb[32:33, :], murstd_row[R, sl],
                                 start=True, stop=True)
                nc.vector.tensor_copy(murstd_bc[:, sl], bcp[:])

            # ---- evidence partials + AllReduce ----
            # ev[v] = sum_t lg[v,t]*rstd[t] - rve[v]*sum_t murstd[t]
            ev_acc = pc.tile([V, TB], F32)
            rv10 = pc.tile([V, 1], F32)
            junk10 = pc.tile([V, T], BF16)
            nc.vector.scalar_tensor_tensor(
                out=junk10[:], in0=murstd_bc[0:V, :],
                scalar=p32_sb[0:V, PC_RVE:PC_RVE + 1],
                in1=murstd_bc[0:V, :], op0=OP.mult, op1=OP.bypass,
                accum_out=rv10[:])
            lgjunk = pc.tile([V, 512], F32)
            for t in range(TB):
                sl = slice(t * 512, (t + 1) * 512)
                nc.vector.scalar_tensor_tensor(
                    out=lgjunk[:], in0=lgs[t][0:V, :], scalar=1.0,
                    in1=rstd_bc[0:V, sl], op0=OP.mult, op1=OP.mult,
                    accum_out=ev_acc[:, t:t + 1])
            ev_sb = pc.tile([V, 1], F32)
            nc.vector.tensor_add(ev_sb[:], ev_acc[:, 0:1], ev_acc[:, 1:2])
            nc.vector.tensor_sub(ev_sb[:], ev_sb[:], rv10[:])

            cc_in = pdram.tile([V, 1], F32)
            cc_out = pdram.tile([V, 1], F32)
            nc.gpsimd.dma_start(out=cc_in[:], in_=ev_sb[:])
            nc.gpsimd.collective_compute(
                "AllReduce", OP.add,
                replica_groups=[[0, 1], [2, 3], [4, 5], [6, 7]],
                ins=[cc_in.opt()], outs=[cc_out.opt()])
            cc_sb = pc.tile([V, 1], F32)
            nc.gpsimd.dma_start(out=cc_sb[:], in_=cc_out[:])

            # ---- belief propagation (tiny; overlaps the main stream) ----
            SIG_C = (0.2499968877665068, -0.020805674064028827,
                     2.0168972875466143e-03, -1.499637664404622e-04)
            SIG3 = (0.24945, -0.0187)

            def emit_sigmoid_poly(out, x, tag):
                c1, c3, c5, c7 = SIG_C
                x2p = pc.tile([V, 1], F32, name=f"sx2_{tag}")
                nc.vector.tensor_mul(x2p[:], x[:], x[:])
                p = pc.tile([V, 1], F32, name=f"sp_{tag}")
                nc.vector.tensor_scalar(p[:], x2p[:], c7, c5, op0=OP.mult,
                                        op1=OP.add)
                nc.vector.tensor_mul(p[:], p[:], x2p[:])
                nc.vector.tensor_scalar(p[:], p[:], c3, None, op0=OP.add)
                nc.vector.tensor_mul(p[:], p[:], x2p[:])
                nc.vector.tensor_scalar(p[:], p[:], c1, None, op0=OP.add)
                nc.vector.tensor_mul(p[:], p[:], x[:])
                nc.vector.tensor_scalar(out[:], p[:], 0.5, None, op0=OP.add)

            bp = {}

            def emit_bp_pre():
                ev_arg = pc.tile([V, 1], F32)
                nc.vector.tensor_scalar_mul(ev_arg[:], cc_sb[:], 1.0 / S)
                nc.vector.tensor_add(ev_arg[:], ev_arg[:],
                                     p32_sb[0:V, PC_BVE:PC_BVE + 1])
                ev0 = pc.tile([V, 1], F32)
                emit_sigmoid_poly(ev0, ev_arg, "ev")
                m1 = pc.tile([V, 1], F32)
                nc.vector.tensor_scalar(m1[:], ev0[:], 0.1, None, op0=OP.is_gt)
                mask = pc.tile([V, 1], F32)
                nc.vector.tensor_scalar(mask[:], ev0[:], 0.9, None,
                                        op0=OP.is_lt)
                nc.vector.tensor_mul(mask[:], mask[:], m1[:])
                nc.vector.tensor_scalar(mask[:], mask[:],
                                        p32_sb[0:V, PC_HASP:PC_HASP + 1],
                                        None, op0=OP.mult)
                probs = pc.tile([V, 1], F32)
                nc.vector.tensor_copy(probs[:], ev0[:])
                bp.update(mask=mask, probs=probs)

            def emit_bp_iter(it):
                mask, probs = bp["mask"], bp["probs"]
                lhsT = pc.tile([V, V], BF16, name=f"lhsT{it}")
                nc.vector.tensor_scalar(lhsT[:],
                                        p32_sb[0:V, PC_PFT:PC_PFT + V],
                                        probs[:, 0:1], None, op0=OP.mult)
                pe_ps = pev.tile([V, 512], F32, tag="lg", bufs=2,
                                 name=f"pe{it}")
                nc.tensor.matmul(pe_ps[:], lhsT[:],
                                 p16_sb[0:V, PB_VAR:PB_VAR + D4],
                                 start=True, stop=True)
                pe_sb = pc.tile([V, D4], F32, tag="bscr", bufs=4,
                                name=f"pe_sb{it}")
                nc.vector.tensor_copy(pe_sb[:], pe_ps[:])
                bscr = pc.tile([V, D4], F32, tag="bscr", bufs=4,
                               name=f"bscr{it}")
                dot = pc.tile([V, 1], F32, name=f"dot{it}")
                nc.vector.scalar_tensor_tensor(
                    out=bscr[:], in0=pe_sb[:], scalar=1.0,
                    in1=p32_sb[0:V, PC_CPT:PC_CPT + D4],
                    op0=OP.mult, op1=OP.mult, accum_out=dot[:])
                bscr2 = pc.tile([V, D4], F32, tag="bscr", bufs=4,
                                name=f"bscr2{it}")
                sqa = pc.tile([V, 1], F32, name=f"sqa{it}")
                nc.vector.scalar_tensor_tensor(
                    out=bscr2[:], in0=pe_sb[:], scalar=1.0, in1=pe_sb[:],
                    op0=OP.mult, op1=OP.mult, accum_out=sqa[:])
                nc.scalar.activation(sqa[:], sqa[:], AF.Sqrt, bias=eps_pn[:])
                ipn = pc.tile([V, 1], F32, name=f"ipn{it}")
                nc.vector.reciprocal(ipn[:], sqa[:])
                s = pc.tile([V, 1], F32, name=f"s{it}")
                nc.vector.tensor_mul(s[:], dot[:], ipn[:])
                sx2 = pc.tile([V, 1], F32, name=f"3x2_{it}")
                nc.vector.tensor_mul(sx2[:], s[:], s[:])
                sp = pc.tile([V, 1], F32, name=f"3p_{it}")
                nc.vector.tensor_scalar(sp[:], sx2[:], SIG3[1], SIG3[0],
                                        op0=OP.mult, op1=OP.add)
                nc.vector.tensor_mul(sp[:], sp[:], s[:])
                cond = pc.tile([V, 1], F32, name=f"cond{it}")
                nc.vector.tensor_scalar(cond[:], sp[:], 0.5, None, op0=OP.add)
                delta = pc.tile([V, 1], F32, name=f"delta{it}")
                nc.vector.tensor_sub(delta[:], cond[:], probs[:])
                nc.vector.scalar_tensor_tensor(
                    out=probs[:], in0=delta[:], scalar=mask[:, 0:1],
                    in1=probs[:], op0=OP.mult, op1=OP.add)

            def emit_ccol():
                probs_bf = pc.tile([V, 1], BF16)
                nc.vector.tensor_copy(probs_bf[:], bp["probs"][:])
                ccol_ps = pev.tile([128, 512], F32, tag="lg", bufs=2,
                                   name="ccol_ps")
                for c in range(NCH):
                    nc.tensor.matmul(ccol_ps[:, c:c + 1],
                                     p16_sb[0:V, PB_W2T + c * 128:
                                            PB_W2T + (c + 1) * 128],
                                     probs_bf[:], start=True, stop=True)
                ccol_sb = pc.tile([128, NCH], F32)
                nc.vector.tensor_add(ccol_sb[:], ccol_ps[:, 0:NCH],
                                     p32_sb[:, PC_BOUT:PC_BOUT + NCH])
                bp["ccol"] = ccol_sb

            # ---- main stream: t-paired accumulation, mixed bf16 + fp8 DR ----
            pend = {}

            def emit_evict(j, t, acc, stage):
                sl = slice(t * 512, (t + 1) * 512)
                s3 = pst.tile([128, 512], BF16, tag="s3", bufs=3,
                              name=f"s3_{j}_{t}")
                nc.vector.scalar_tensor_tensor(
                    out=s3[:], in0=acc[:], scalar=1.0, in1=rstd_bc[:, sl],
                    op0=OP.mult, op1=OP.mult)
                s4 = pst.tile([128, 512], BF16, tag="s4", bufs=3,
                              name=f"s4_{j}_{t}")
                nc.vector.scalar_tensor_tensor(
                    out=s4[:], in0=murstd_bc[:, sl],
                    scalar=p32_sb[:, PC_NR1 + j:PC_NR1 + j + 1],
                    in1=s3[:], op0=OP.mult, op1=OP.add)
                xb = xbfp[j // 2][:, j % 2, sl]
                if j // 2 >= FOLD_P:
                    nc.vector.scalar_tensor_tensor(
                        out=stage[:, j % 2, sl], in0=xb,
                        scalar=bp["ccol"][:, j:j + 1],
                        in1=s4[:], op0=OP.add, op1=OP.add)
                else:
                    nc.vector.tensor_add(stage[:, j % 2, sl], xb, s4[:])

            def pair_out_ap(p):
                return out_d[p * 256:(p + 1) * 256, :].rearrange(
                    "(i p) t -> p i t", p=128)

            def emit_pair_dma(p, stage, nsp, ring):
                ap = pair_out_ap(p)
                for qq in range(nsp):
                    p0 = qq * (128 // nsp)
                    p1 = p0 + (128 // nsp)
                    eng = ring[qq % len(ring)]
                    eng.dma_start(out=ap[p0:p1], in_=stage[p0:p1])

            def emit_late_out():
                for p in range(FOLD_P):
                    stage = pend.pop(p)
                    nc.scalar.activation(
                        stage[:, 0, :], stage[:, 0, :], AF.Identity,
                        bias=bp["ccol"][:, 2 * p:2 * p + 1])
                    nc.scalar.activation(
                        stage[:, 1, :], stage[:, 1, :], AF.Identity,
                        bias=bp["ccol"][:, 2 * p + 1:2 * p + 2])
                    ring = [nc.scalar] if p % 2 == 0 else [nc.sync]
                    emit_pair_dma(p, stage, 1, ring)

            for j in range(NCH):
                if j % 2 == 0:
                    stage = pst.tile([128, 2, T], BF16, tag="stage", bufs=8,
                                     name=f"stage{j // 2}")
                accs = [pacc.tile([128, 512], F32, tag="acc", bufs=4,
                                  name=f"acc{j}_{t}") for t in range(TB)]
                for hin in range(K_BF):
                    wst = w1bp[j // 2][:, j % 2, hin, :]
                    for t in range(TB):
                        sl = slice(t * 512, (t + 1) * 512)
                        nc.tensor.matmul(
                            accs[t][:], wst, xbfp[hin // 2][:, hin % 2, sl],
                            start=(hin == 0), stop=False)
                for m in range(NDR):
                    p2 = 5 + m
                    q, o = p2 // 2, (p2 % 2) * 2
                    wst = w18_sb[j // 8][:, (j // 2) % 4, j % 2, m, :, :]
                    for t in range(TB):
                        sl = slice(t * 512, (t + 1) * 512)
                        nc.tensor.matmul(
                            accs[t][:], wst, x8q[q][:, o:o + 2, sl],
                            start=False, stop=(m == NDR - 1),
                            perf_mode=PM.DoubleRow)
                    if j == 6 and m == NDR - 1:
                        emit_bp_pre()
                        emit_bp_iter(0)
                for t in range(TB):
                    emit_evict(j, t, accs[t], stage)
                if j % 2 == 1 and j // 2 < FOLD_P:
                    pend[j // 2] = stage
                if j % 2 == 1 and j // 2 == FOLD_P:
                    emit_pair_dma(FOLD_P, stage, 4, [nc.sync, nc.scalar])
                if j < 6:
                    release_w1b(j + 2)
                if j == 0:
                    release_w18g1()
                if 1 <= j <= 3:
                    release_xbf(j + 4)
                if j in (7, 8, 9, 10):
                    emit_bp_iter(j - 6)
                if j == 11:
                    emit_ccol()
                if j == 13:
                    emit_late_out()

    nc.compile()
    return nc


def _host_prep(hidden_states, gamma, beta, W_ve, b_ve, var_emb, cpt_emb,
               W_out, b_out, parents):
    f32 = np.float32
    bf16 = ml_dtypes.bfloat16
    fp8 = ml_dtypes.float8_e4m3
    x = np.asarray(hidden_states, f32).reshape(B * S, H)
    gamma = np.asarray(gamma, f32)
    beta = np.asarray(beta, f32)
    W_ve = np.asarray(W_ve, f32)
    b_ve = np.asarray(b_ve, f32)
    var_emb = np.asarray(var_emb, f32)
    cpt_emb = np.asarray(cpt_emb, f32)
    W_out = np.asarray(W_out, f32)
    b_out = np.asarray(b_out, f32)
    parents = np.asarray(parents)

    W1 = W_out[:, :H]
    W1g = W1 * gamma[None, :]
    # quantized W1g: chunks < K_BF as bf16(64*W), rest fp8(16*W)
    w1b = np.ascontiguousarray(
        (64.0 * W1g).T.reshape(NCH, 128, 8, 2, 128)[:K_BF]
        .transpose(1, 2, 3, 0, 4)).astype(bf16)
    # w18[p, g, q, jj, m, i, c] = fp8(16*W1g[j*128+c, (K_BF+2m+i)*128+p])
    w18f = (16.0 * W1g).T.reshape(NCH, 128, 2, 4, 2, 128)[K_BF:]
    w18f = w18f.reshape(NDR, 2, 128, 2, 4, 2, 128)
    w18 = np.ascontiguousarray(w18f.transpose(2, 3, 4, 5, 0, 1, 6)).astype(fp8)
    # exact column sums of the quantized matrix (true scale)
    wq_eff = np.empty_like(W1g)
    for ch in range(NCH):
        blk = W1g[:, ch * 128:(ch + 1) * 128]
        if ch < K_BF:
            wq_eff[:, ch * 128:(ch + 1) * 128] = \
                (64.0 * blk).astype(bf16).astype(f32) / 64.0
        else:
            wq_eff[:, ch * 128:(ch + 1) * 128] = \
                (16.0 * blk).astype(fp8).astype(f32) / 16.0
    r1 = wq_eff.sum(axis=1)                                  # [H]
    Wveg = W_ve * gamma[None, :]
    wve8 = np.zeros((128, NCH // 2 + 1, 2, 48), fp8)
    wq = (16.0 * Wveg.T).astype(fp8)                         # [H, V]
    wve8[:, :8, :, :V] = wq.reshape(NCH // 2, 2, 128, V).transpose(2, 0, 1, 3)
    wve8[:, :8, :, 32] = np.float32(16.0)                    # 64*sum(x) row
    wve8[:, 8, :, 32] = np.float32(1.0)                      # sq ones column
    rve = (wq.astype(f32) / 16.0).sum(axis=0)                # [V]

    p32 = np.zeros((128, PC_N), f32)
    p32[:, PC_NR1:PC_NR1 + NCH] = (-r1).reshape(NCH, 128).T
    p32[:, PC_BOUT:PC_BOUT + NCH] = (b_out + W1 @ beta).reshape(NCH, 128).T
    icn = 1.0 / np.maximum(np.sqrt((cpt_emb * cpt_emb).sum(axis=1)), 1e-8)
    p32[:V, PC_CPT:PC_CPT + D4] = cpt_emb * icn[:, None]
    p32[:V, PC_PFT:PC_PFT + V] = parents.T.astype(f32)
    p32[:V, PC_RVE] = rve
    p32[:V, PC_BVE] = b_ve + W_ve @ beta
    p32[:V, PC_HASP] = (parents.sum(axis=1) > 0).astype(f32)
    p16 = np.zeros((128, PB_N), bf16)
    p16[:V, PB_W2T:PB_W2T + H] = np.ascontiguousarray(W_out[:, H:].T)
    p16[:V, PB_VAR:PB_VAR + D4] = var_emb.astype(bf16)

    shared = dict(w1b=w1b, w18=w18, wve8=wve8, p32=p32, p16=p16)
    in_maps = []
    for c in range(N_CORES):
        xs = x[c * T:(c + 1) * T, :]
        xbfT = np.ascontiguousarray(
            xs.T.reshape(NCH, 128, T).transpose(1, 0, 2)).astype(bf16)
        x8T = np.ascontiguousarray(
            (4.0 * xs).T.reshape(NCH, 128, T).transpose(1, 0, 2)).astype(fp8)
        in_maps.append(dict(shared, xbfT=xbfT, x8T=x8T))
    return in_maps


def kernel(**inputs):
    global _PROG
    if _PROG is None:
        _PROG = build_program()
    nc = _PROG
    in_maps = _host_prep(**inputs)
    res = run_bass_kernel_spmd(nc, in_maps, list(range(N_CORES)))
    out = np.empty((B * S, H), np.float32)
    for c in range(N_CORES):
        out[c * T:(c + 1) * T, :] = \
            np.asarray(res.results[c]["outT"]).astype(np.float32).T
    return out.reshape(B, S, H)
